# revision 38
# baseline (speedup 1.0000x reference)
"""Trainium2 Bass kernel for nn_Attention_12000138625343.

Full multi-head attention layer (B=2, S=2048, E=1024, H=16, hd=64, interleaved
RoPE on q/k, non-causal softmax) run tensor-parallel over 8 NeuronCores:

  - heads sharded 2-per-core (w1 columns / qkv projection sharded),
  - x replicated, passed pre-transposed [E, B*S] in bf16 so the contraction
    dim lands on SBUF partitions,
  - all matmul operands in bf16 (PSUM accumulation stays fp32): halves HBM
    traffic, SBUF footprint, and the A2A payload,
  - bulk loads ride the gpsimd SWDGE queue (descriptor-gen ~1us each, the
    transfers fan out across the DMA-engine pool); HWDGE queues serialize
    bulk data so they only carry small sends,
  - scores computed transposed [k, q]; the two heads' K=64 score matmuls are
    packed into disjoint PE row-groups, one exp instruction covers both
    heads' [128, 1024] PSUM block,
  - attn@v accumulates rolling per k-chunk with a ones-column appended to v
    producing the softmax denominator in row 64,
  - the softmax divide happens AFTER the A2A on the receive side: each core
    sends raw o^T plus the denominator row per head; the receiver
    reciprocates all 16 denominators in one partition-parallel DVE op,
    broadcasts via a DRAM-bounce DMA, and multiplies once per half. This
    keeps the entire divide chain off the kernel tail,
  - AllToAll halves are "diagonal": half A = {qt0, qt3}, half B = {qt1, qt2}
    (each covers all 8 destination cores). Half B completes at 3/4 of the
    batch so its collective and projection overlap the remaining attention
    units; only half A's 0.26 MB collective remains at the batch end. Cores
    0-3 receive row-block 0 from half A, cores 4-7 row-block 1 (host gather
    compensates),
  - batch-1 qkv projection / output projection matmul chains are dribbled
    into the attention k-chunk loop so the in-order PE stream never starves
    the exp pipeline for long,
  - each core owns 2 x 128 rows of each batch; host reassembles.
"""

import math

import numpy as np

import concourse.bass as bass
import concourse.mybir as mybir
import concourse.tile as tile
from concourse import bacc
from concourse.bass_utils import run_bass_kernel_spmd
from concourse.masks import make_identity

B, S, E, H = 2, 2048, 1024, 16
HD = E // H  # 64
BASE = 10000.0
N_CORES = 8
HPC = H // N_CORES       # heads per core = 2
R = B * S                # 4096 flattened rows
RT = 512                 # rows per r-tile
NEC = E // 128           # 8 e-chunks of 128
QT = 512                 # q columns per q-tile
N_QT = S // QT           # 4 q-tiles per batch
KC = 128                 # k rows per k-chunk
N_KC = S // KC           # 16 k-chunks per batch
RPB = S // N_CORES       # rows per core per batch = 256

F32 = mybir.dt.float32
BF16 = mybir.dt.bfloat16
FP8 = mybir.dt.float8e4
EXPF = mybir.ActivationFunctionType.Exp
DROW = mybir.MatmulPerfMode.DoubleRow
# softmax numerator/denominator are both scaled by 1/EXPC (ratio unchanged);
# keeps exp(s)/EXPC <= 240 (TRN fp8e4 max) for any realistic score
EXPC = 4.0

# diagonal A2A halves: each covers all 8 destination cores
HALF = {0: "A", 1: "B", 2: "B", 3: "A"}

_COMPILED = {}


def _build_nc():
    nc = bacc.Bacc("TRN2", target_bir_lowering=False, debug=False,
                   num_devices=N_CORES)

    xT = nc.dram_tensor("xT", [E, R], BF16, kind="ExternalInput").ap()
    wqT = nc.dram_tensor("wqT", [E, 128], BF16, kind="ExternalInput").ap()
    wkT = nc.dram_tensor("wkT", [E, 128], BF16, kind="ExternalInput").ap()
    wvT = nc.dram_tensor("wvT", [E, 128], BF16, kind="ExternalInput").ap()
    w2T = nc.dram_tensor("w2T", [E, E], BF16, kind="ExternalInput").ap()
    cosT = nc.dram_tensor("cosT", [128, S], F32, kind="ExternalInput").ap()
    sinT = nc.dram_tensor("sinT", [128, S], F32, kind="ExternalInput").ap()
    p2T = nc.dram_tensor("p2T", [128, 128], BF16, kind="ExternalInput").ap()
    out = nc.dram_tensor("out", [2 * RPB, E], F32, kind="ExternalOutput").ap()

    with tile.TileContext(nc) as tc:
        _emit(tc, nc, xT, wqT, wkT, wvT, w2T, cosT, sinT, p2T, out)
    nc.compile()
    return nc


def _emit(tc, nc, xT, wqT, wkT, wvT, w2T, cosT, sinT, p2T, out):
    import contextlib
    ctx = contextlib.ExitStack()
    consts = ctx.enter_context(tc.tile_pool(name="consts", bufs=1))
    xtp = ctx.enter_context(tc.tile_pool(name="xtp", bufs=2))
    qkp = ctx.enter_context(tc.tile_pool(name="qkp", bufs=1))
    rawp = ctx.enter_context(tc.tile_pool(name="rawp", bufs=2))
    tmpp = ctx.enter_context(tc.tile_pool(name="tmpp", bufs=2))
    vp = ctx.enter_context(tc.tile_pool(name="vp", bufs=1))
    pp = ctx.enter_context(tc.tile_pool(name="pp", bufs=7))
    smallp = ctx.enter_context(tc.tile_pool(name="smallp", bufs=2))
    dramp = ctx.enter_context(tc.tile_pool(name="dramp", bufs=1, space="DRAM"))
    # PSUM budget (8 banks): qkv-shared 2 + sps 2 x 2 + av 2 = 8
    ps_qkv = ctx.enter_context(tc.tile_pool(name="ps_qkv", bufs=2, space="PSUM"))
    ps_sps = ctx.enter_context(tc.tile_pool(name="ps_sps", bufs=2, space="PSUM"))
    ps_av = ctx.enter_context(tc.tile_pool(name="ps_av", bufs=2, space="PSUM"))

    # ---- bulk input loads on the gpsimd SWDGE queue (parallel transfers);
    # with xt bufs=8 nothing waits in-FIFO, so the collective triggers
    # emitted later are never delayed ----
    wq_all = consts.tile([128, NEC, 128], BF16, tag="wq", name="wq_all")
    nc.sync.dma_start(out=wq_all[:], in_=wqT.rearrange("(c p) f -> p c f", p=128))

    xts = []
    for rt in range(2 * N_QT):
        t = xtp.tile([128, NEC, RT], BF16, tag="xt", bufs=8, name=f"xt_{rt}")
        nc.gpsimd.dma_start(
            out=t[:],
            in_=xT.rearrange("(c p) r -> p c r", p=128)[:, :, rt * RT:(rt + 1) * RT])
        xts.append(t)
        if rt == 0:
            wk_all = consts.tile([128, NEC, 128], BF16, tag="wk", name="wk_all")
            nc.gpsimd.dma_start(
                out=wk_all[:], in_=wkT.rearrange("(c p) f -> p c f", p=128))
            wv_all = consts.tile([128, NEC, 128], BF16, tag="wv", name="wv_all")
            nc.gpsimd.dma_start(
                out=wv_all[:], in_=wvT.rearrange("(c p) f -> p c f", p=128))
            p2_sb = consts.tile([128, 128], BF16, tag="p2", name="p2_sb")
            nc.gpsimd.dma_start(out=p2_sb[:], in_=p2T[:, :])
        if rt == 3:
            # after the batch-0 x tiles: rope tables (first needed ~10us in),
            # before batch-1's x tiles and w2 (needed much later)
            cos_sb = consts.tile([128, S], F32, tag="cos", name="cos_sb")
            nc.gpsimd.dma_start(out=cos_sb[:], in_=cosT[:, :])
            sin_sb = consts.tile([128, S], F32, tag="sin", name="sin_sb")
            nc.gpsimd.dma_start(out=sin_sb[:], in_=sinT[:, :])
    # w2: 2 MB, overlaps the batch-0 qkv/attention stretch
    w2_all = consts.tile([128, NEC, E], BF16, tag="w2", name="w2_all")
    nc.gpsimd.dma_start(out=w2_all[:], in_=w2T.rearrange("(c p) f -> p c f", p=128))

    ones_f32 = consts.tile([128, 64], F32, tag="ones32", name="ones_f32")
    nc.vector.memset(ones_f32[:], 1.0)
    ones_rb = consts.tile([1, 64], BF16, tag="onesrb", name="ones_rb")
    nc.vector.tensor_copy(ones_rb[:], ones_f32[0:1, 0:64])
    id_sb = consts.tile([128, 128], F32, tag="idm", name="id_sb")
    make_identity(nc, id_sb[:])

    # A2A buffers, one pair per (batch, half): dest core j's chunk is
    # [2 heads, 65 rows (64 o^T + denominator), 128 s-cols]
    send_d = {(b, hf): dramp.tile([N_CORES, HPC, 65, 128], BF16,
                                  name=f"send{b}{hf}")
              for b in range(B) for hf in ("A", "B")}
    recv_d = {(b, hf): dramp.tile([N_CORES, HPC, 65, 128], BF16,
                                  name=f"recv{b}{hf}")
              for b in range(B) for hf in ("A", "B")}

    qT_sb, kT_sb, v_sb = {}, {}, {}

    def qkv_chains(rt):
        """Return a list of closures, each emitting one matmul chain (+ its
        epilogue) for r-tile rt. Callers dribble these between attention
        steps to keep the in-order PE stream dense but never monolithic."""
        b, st = rt // N_QT, (rt % N_QT) * RT
        xt = xts[rt]

        if b not in qT_sb:
            qT_sb[b] = qkp.tile([128, S], BF16, tag=f"qT{b}", name=f"qT{b}")
            kT_sb[b] = qkp.tile([128, S], BF16, tag=f"kT{b}", name=f"kT{b}")

        def qk_chain(kind, w_all, dst):
            state = {}
            def emit_a():
                acc = ps_qkv.tile([128, RT], F32, tag="qkv",
                                  name=f"{kind}acc{rt}")
                for ec in range(4):
                    nc.tensor.matmul(acc[:], w_all[:, ec, :], xt[:, ec, :],
                                     start=(ec == 0), stop=False)
                state["acc"] = acc
            def emit_b():
                acc = state.pop("acc")
                for ec in range(4, NEC):
                    nc.tensor.matmul(acc[:], w_all[:, ec, :], xt[:, ec, :],
                                     start=False, stop=(ec == NEC - 1))
                raw = rawp.tile([128, RT], BF16, tag="raw",
                                name=f"{kind}raw{rt}")
                # DVE eviction: keep the Scalar engine free for exp, which
                # paces the attention phases these chains dribble into
                nc.vector.tensor_copy(raw[:], acc[:])
                rot = ps_qkv.tile([128, RT], F32, tag="qkv",
                                  name=f"{kind}rot{rt}")
                nc.tensor.matmul(rot[:], p2_sb[:], raw[:], start=True, stop=True)
                t1 = tmpp.tile([128, RT], F32, tag="ropet", name=f"{kind}t1_{rt}")
                nc.vector.tensor_mul(t1[:], raw[:], cos_sb[:, st:st + RT])
                t2 = tmpp.tile([128, RT], F32, tag="ropet", name=f"{kind}t2_{rt}")
                nc.vector.tensor_mul(t2[:], rot[:], sin_sb[:, st:st + RT])
                nc.vector.tensor_add(dst[:, st:st + RT], t1[:], t2[:])
            return [emit_a, emit_b]

        vstate = {}

        def v_head_chain(half):
            # v^T = wv.T @ x computed at full rate (N=512), half the e-chunks
            # per pop; the PE transpose in v_tail_chain flips it back to the
            # [k, hd] layout attn@v needs.
            def emit():
                if half == 0:
                    vacc = ps_qkv.tile([128, RT], F32, tag="qkv",
                                       name=f"vTacc{rt}")
                    vstate["ps"] = vacc
                vacc = vstate["ps"]
                for ec in range(4 * half, 4 * half + 4):
                    nc.tensor.matmul(vacc[:], wv_all[:, ec, :], xt[:, ec, :],
                                     start=(ec == 0), stop=(ec == NEC - 1))
                if half == 1:
                    vts = rawp.tile([128, RT], F32, tag="raw",
                                    name=f"vts{rt}")
                    nc.vector.tensor_copy(vts[:], vstate.pop("ps")[:])
                    vstate["sb"] = vts
            return emit

        def v_tail_chain(pair):
            def emit():
                vts = vstate["sb"]
                for sub in (2 * pair, 2 * pair + 1):
                    vtr = ps_qkv.tile([128, 128], F32, tag="qkv",
                                      name=f"vtr{rt}_{sub}")
                    nc.tensor.transpose(
                        vtr[:], vts[:, sub * 128:(sub + 1) * 128], id_sb[:])
                    kc = (rt % N_QT) * 4 + sub
                    for h in range(HPC):
                        vt = vp.tile([128, 65], BF16, tag=f"v{b}{h}{kc}",
                                     name=f"v{b}{h}{kc}")
                        nc.vector.tensor_copy(vt[:, 0:64],
                                              vtr[:, h * 64:(h + 1) * 64])
                        nc.vector.tensor_copy(vt[:, 64:65], ones_f32[:, 0:1])
                        v_sb[(b, h, kc)] = vt
            return emit

        return qk_chain("q", wq_all, qT_sb[b]) + \
               qk_chain("k", wk_all, kT_sb[b]) + \
               [v_head_chain(0), v_head_chain(1),
                v_tail_chain(0), v_tail_chain(1)]

    def proj_chains(b, hf):
        """Output projection for my 128 rows of (batch b, diagonal half hf).
        The recv load + softmax divide are emitted lazily by the first chain
        so they never precede the collective's emission."""
        state0 = {}
        def get_odv():
            if "odv" not in state0:
                # o^T rows: e-row within source chunk c is h*64+p
                recv_sb = xtp.tile([128, NEC, 128], BF16, tag="recv", bufs=2,
                                   name=f"recv{b}{hf}")
                for h in range(HPC):
                    nc.gpsimd.dma_start(
                        out=recv_sb[h * 64:(h + 1) * 64, :, :],
                        in_=recv_d[(b, hf)][:, h, 0:64, :].rearrange(
                            "c p r -> p c r"))
                # denominator rows, one partition per (source, head)
                dn = smallp.tile([16, 128], BF16, tag="dn", name=f"dn{b}{hf}")
                nc.gpsimd.dma_start(
                    out=dn[:],
                    in_=recv_d[(b, hf)][:, :, 64:65, :].rearrange(
                        "c h p r -> (c h p) r"))
                # all 16 reciprocals in one partition-parallel op (~0.9us)
                rcp16 = smallp.tile([16, 128], BF16, tag="rcp16",
                                    name=f"rcp16{b}{hf}")
                with nc.allow_low_precision(reason="bf16 1/denominator"):
                    nc.vector.reciprocal(rcp16[:], dn[:])
                # stage 1/d in DRAM as [h, c*128+r] so every broadcast read
                # below is one contiguous 2 KB row, not strided 256 B pieces
                rcp_dr = dramp.tile([HPC, NEC * 128], BF16, tag="rcpd",
                                    bufs=2, name=f"rcpd{b}{hf}")
                st_ap = bass.AP(tensor=rcp_dr.tensor, offset=rcp_dr.offset,
                                ap=[[128, N_CORES], [NEC * 128, HPC],
                                    [1, 128]])
                nc.sync.dma_start(out=st_ap, in_=rcp16[:])
                odv = xtp.tile([128, NEC, 128], BF16, tag="odv", bufs=2,
                               name=f"odv{b}{hf}")
                if b == 1:
                    # tail halves: the attention stream is over, so ps_sps
                    # and the PE are free — broadcast 1/d with one K=1
                    # matmul per head (the replicating-DMA bounce costs
                    # ~10us of strided small reads; this is ~1us)
                    dnr = []
                    for h in range(HPC):
                        t = smallp.tile([1, NEC * 128], BF16, tag=f"dnr{h}",
                                        name=f"dnr{b}{hf}{h}")
                        nc.sync.dma_start(out=t[:], in_=rcp_dr[h:h + 1, :])
                        dnr.append(t)
                    bc_ps = ps_sps.tile([128, NEC * 128], F32, tag="sps",
                                        name=f"bcps{b}{hf}")
                    for h in range(HPC):
                        for half in range(2):
                            sl = slice(half * 512, (half + 1) * 512)
                            nc.tensor.matmul(bc_ps[h * 64:(h + 1) * 64, sl],
                                             ones_rb[:], dnr[h][:, sl],
                                             start=True, stop=True)
                    nc.vector.tensor_mul(
                        odv[:], recv_sb[:],
                        bc_ps[:].rearrange("p (c r) -> p c r", c=NEC))
                else:
                    # mid-stream halves: latency is hidden by the attention
                    # stream — broadcast via the DRAM bounce (each partition
                    # reads its head's full 2 KB row)
                    bcast = smallp.tile([128, NEC, 128], BF16, tag="bcast",
                                        name=f"bcast{b}{hf}")
                    for h in range(HPC):
                        bc_ap = bass.AP(
                            tensor=rcp_dr.tensor,
                            offset=rcp_dr.offset + h * NEC * 128,
                            ap=[[0, 64], [128, NEC], [1, 128]])
                        nc.sync.dma_start(out=bcast[h * 64:(h + 1) * 64, :, :],
                                          in_=bc_ap)
                    nc.vector.tensor_mul(odv[:], recv_sb[:], bcast[:])
                state0["odv"] = odv
            return state0["odv"]
        chains = []
        rblk = {"A": 0, "B": 1}[hf]     # for cores 0-3; host swaps for 4-7
        for ft in range(2):
            state = {}
            def emit_a(ft=ft, state=state):
                odv = get_odv()
                # qkv psum tag: free during attention (projection is done)
                ops = ps_qkv.tile([128, 512], F32, tag="qkv",
                                  name=f"ops{b}_{rblk}_{ft}")
                for ec in range(4):
                    nc.tensor.matmul(
                        ops[:],
                        odv[:, ec, :],
                        w2_all[:, ec, ft * 512:(ft + 1) * 512],
                        start=(ec == 0), stop=False)
                state["ops"] = ops
            def emit_b(ft=ft, state=state):
                odv = get_odv()
                ops = state.pop("ops")
                for ec in range(4, NEC):
                    nc.tensor.matmul(
                        ops[:],
                        odv[:, ec, :],
                        w2_all[:, ec, ft * 512:(ft + 1) * 512],
                        start=False, stop=(ec == NEC - 1))
                ot = tmpp.tile([128, 512], F32, tag="ropet",
                               name=f"ot{b}_{rblk}_{ft}")
                if b == 1 and hf == "A":
                    # kernel tail: exp stream is over, ACT is free
                    nc.scalar.copy(ot[:], ops[:])
                else:
                    # runs during an attention stretch where exp keeps ACT
                    # busy: evict on DVE
                    nc.vector.tensor_copy(ot[:], ops[:])
                ob = 2 * b + rblk
                nc.sync.dma_start(
                    out=out[ob * 128:(ob + 1) * 128,
                            ft * 512:(ft + 1) * 512],
                    in_=ot[:])
            chains.append(emit_a)
            chains.append(emit_b)
        return chains

    def emit_stage(b, qt, avs):
        """Evict the attn@v accumulator (o^T raw + denominator row) straight
        into the A2A send buffer; the divide happens on the receive side."""
        last = (b == B - 1 and qt == N_QT - 1)
        hf = HALF[qt]
        jbase = 4 * (qt % 2)
        for h in range(HPC):
            # evict immediately: releases the PSUM slot so the next q-tile's
            # attn@v never waits
            oraw = smallp.tile([65, QT], BF16, tag="oraw",
                               name=f"oraw{b}{h}{qt}")
            if last:
                nc.scalar.copy(oraw[:], avs[h][:])
            else:
                # exp paces the attention stream: keep evictions off ACT
                nc.vector.tensor_copy(oraw[:], avs[h][:])
            # at the kernel tail these sends gate the final A2A: split them
            # across both HWDGE queues (ACT is idle there)
            eng = nc.scalar if (last and h == 1) else nc.sync
            for jj in range(4):
                eng.dma_start(
                    out=send_d[(b, hf)][jbase + jj, h, :, :],
                    in_=oraw[:, jj * 128:(jj + 1) * 128])

    def emit_attention_batch(b, dribble):
        """All 4 q-tiles of a batch as one rolling pipeline over 64+LAG
        (qt, kc) units: scores+exp lead, attn@v trails by LAG units, the
        staging fires as each q-tile's accumulation completes. One dribble
        chain (qkv projection / output projection) is popped every other
        unit to keep the in-order PE stream dense."""
        scale = 1.0 / math.sqrt(HD)
        NU = N_QT * N_KC
        LAG = 5
        pts = {}
        avs = {}
        for u in range(NU + LAG):
            if u < NU:
                qt, kc = divmod(u, N_KC)
                if kc == 0:
                    avs[qt] = [ps_av.tile([65, QT], F32, tag="av",
                                          name=f"av{b}{h}{qt}")
                               for h in range(HPC)]
                sps = ps_sps.tile([128, 2 * QT], F32, tag="sps",
                                  name=f"s{b}{qt}_{kc}")
                for h in range(HPC):
                    hof = h * 64
                    nc.tensor.matmul(
                        sps[:, h * QT:(h + 1) * QT],
                        kT_sb[b][hof:hof + 64, kc * KC:(kc + 1) * KC],
                        qT_sb[b][hof:hof + 64, qt * QT:(qt + 1) * QT],
                        start=True, stop=True)
                pt = pp.tile([128, 2 * QT], BF16, tag="p", name=f"p{b}{qt}_{kc}")
                nc.scalar.activation(pt[:], sps[:], EXPF, scale=scale)
                pts[u] = pt
            if u >= LAG:
                j = u - LAG
                qt2, kc2 = divmod(j, N_KC)
                for h in range(HPC):
                    nc.tensor.matmul(avs[qt2][h][:], v_sb[(b, h, kc2)][:],
                                     pts[j][:, h * QT:(h + 1) * QT],
                                     start=(kc2 == 0), stop=(kc2 == N_KC - 1))
                del pts[j]
                if kc2 == N_KC - 1:
                    emit_stage(b, qt2, avs.pop(qt2))
                    if qt2 == 2:
                        emit_a2a(b, "B")
            # one chain per two units, ramping up near the end so no
            # backlog remains to run as a monolithic lump afterwards
            if dribble and dribble[0][0] <= u and (
                    u % 2 == 1 or 2 * len(dribble) >= (NU + LAG - u)):
                dribble.pop(0)[1]()

    def emit_a2a(b, hf):
        nc.gpsimd.collective_compute(
            "AllToAll", mybir.AluOpType.bypass,
            replica_groups=[list(range(N_CORES))],
            ins=[send_d[(b, hf)].opt()], outs=[recv_d[(b, hf)].opt()])

    # ---------------- emission ----------------
    for rt in range(N_QT):             # batch-0 projection: pure PE stretch
        for chain in qkv_chains(rt):
            chain()
    # warm the collective path (cold-start ~8us); emitted here so the wait on
    # the gpsimd queue never delays the critical first x/weight loads
    cwu_s = dramp.tile([N_CORES, 8], F32, tag="cwus", name="cwu_s")
    cwu_r = dramp.tile([N_CORES, 8], F32, tag="cwur", name="cwu_r")
    nc.sync.dma_start(out=cwu_s.rearrange("c r -> (c r)")[None, :],
                      in_=ones_f32[0:1, 0:64])
    nc.gpsimd.collective_compute(
        "AllToAll", mybir.AluOpType.bypass,
        replica_groups=[list(range(N_CORES))],
        ins=[cwu_s.opt()], outs=[cwu_r.opt()])

    # batch-0 attention with batch-1 qkv dribbled in; A2A(0,B) fires at 3/4
    dribble = [(1, c) for rt in range(N_QT, 2 * N_QT) for c in qkv_chains(rt)]
    emit_attention_batch(0, dribble)
    for _, chain in dribble:
        chain()
    del dribble[:]
    emit_a2a(0, "A")                   # fires at batch-0 end

    # batch-1 attention: batch-0 projections early (both its A2As have
    # landed). Batch-1's own projections run post-loop: their PE matmuls
    # wait on collectives, and dribbling them would fence the in-order PE
    # stream mid-attention. A2A(1,A) is emitted first so its trigger fires
    # the moment the qt3 sends land.
    dribble = [(5, c) for c in proj_chains(0, "B")]
    dribble += [(40, c) for c in proj_chains(0, "A")]
    emit_attention_batch(1, dribble)
    for _, chain in dribble:
        chain()
    emit_a2a(1, "A")
    for chain in proj_chains(1, "B"):
        chain()
    for chain in proj_chains(1, "A"):
        chain()
    ctx.close()


def _host_prep(x, w1, w2):
    import ml_dtypes
    bf16 = ml_dtypes.bfloat16
    x = np.asarray(x, dtype=np.float32)
    w1 = np.asarray(w1, dtype=np.float32)
    w2 = np.asarray(w2, dtype=np.float32)

    xT = np.ascontiguousarray(x.reshape(R, E).T.astype(bf16))      # [E, R]
    w2T = np.ascontiguousarray(w2.T.astype(bf16))                  # [E, E]

    theta = 1.0 / (BASE ** (np.arange(0, HD, 2, dtype=np.float32) / HD))
    enc = np.arange(S, dtype=np.float32)[:, None] * theta[None, :]
    enc = np.repeat(enc, 2, axis=-1)                      # [s, 64]
    cos1 = np.cos(enc).T.astype(np.float32)               # [64, S]
    sin1 = np.sin(enc).T.astype(np.float32)
    cosT = np.ascontiguousarray(np.concatenate([cos1, cos1], axis=0))
    sinT = np.ascontiguousarray(np.concatenate([sin1, sin1], axis=0))

    m64 = np.zeros((HD, HD), dtype=np.float32)
    for i in range(HD // 2):
        m64[2 * i, 2 * i + 1] = -1.0
        m64[2 * i + 1, 2 * i] = 1.0
    m128 = np.zeros((128, 128), dtype=np.float32)
    m128[:64, :64] = m64
    m128[64:, 64:] = m64
    p2T = np.ascontiguousarray(m128.T.astype(bf16))

    in_maps = []
    for c in range(N_CORES):
        hA, hB = HPC * c, HPC * c + 1
        def rows(base):
            return np.concatenate(
                [w1[base + hA * HD: base + (hA + 1) * HD, :],
                 w1[base + hB * HD: base + (hB + 1) * HD, :]], axis=0)
        in_maps.append({
            "xT": xT,
            "wqT": np.ascontiguousarray(rows(0).T.astype(bf16)),
            "wkT": np.ascontiguousarray(rows(E).T.astype(bf16)),
            "wvT": np.ascontiguousarray(rows(2 * E).T.astype(bf16)),
            "w2T": w2T,
            "cosT": cosT,
            "sinT": sinT,
            "p2T": p2T,
        })
    return in_maps


def kernel(x, w1, w2, _trace=False):
    if "nc" not in _COMPILED:
        _COMPILED["nc"] = _build_nc()
    nc = _COMPILED["nc"]
    in_maps = _host_prep(x, w1, w2)
    res = run_bass_kernel_spmd(nc, in_maps, core_ids=list(range(N_CORES)),
                               trace=_trace)
    _COMPILED["last_result"] = res
    # core c returns [512, E] as four 128-row blocks written per (batch,
    # half): [b0 A, b0 B, b1 A, b1 B]. Half A carries s-rows 128c for cores
    # 0-3 but 1024+128c for cores 4-7 (diagonal halves); B is the opposite.
    full = np.empty((B, S, E), dtype=np.float32)
    for c in range(N_CORES):
        blk = res.results[c]["out"]
        lo, hi = (0, 1) if c < 4 else (1, 0)   # blk index carrying s=128c
        for b in range(B):
            full[b, 128 * c:128 * (c + 1)] = blk[(2 * b + lo) * 128:
                                                 (2 * b + lo + 1) * 128]
            full[b, 1024 + 128 * c:1024 + 128 * (c + 1)] = \
                blk[(2 * b + hi) * 128:(2 * b + hi + 1) * 128]
    return full


# revision 39
# speedup vs baseline: 1.0106x; 1.0106x over previous
"""Trainium2 Bass kernel for nn_Attention_12000138625343.

Full multi-head attention layer (B=2, S=2048, E=1024, H=16, hd=64, interleaved
RoPE on q/k, non-causal softmax) run tensor-parallel over 8 NeuronCores:

  - heads sharded 2-per-core (w1 columns / qkv projection sharded),
  - x replicated, passed pre-transposed [E, B*S] in bf16 so the contraction
    dim lands on SBUF partitions,
  - all matmul operands in bf16 (PSUM accumulation stays fp32): halves HBM
    traffic, SBUF footprint, and the A2A payload,
  - bulk loads ride the gpsimd SWDGE queue (descriptor-gen ~1us each, the
    transfers fan out across the DMA-engine pool); HWDGE queues serialize
    bulk data so they only carry small sends,
  - scores computed transposed [k, q]; the two heads' K=64 score matmuls are
    packed into disjoint PE row-groups, one exp instruction covers both
    heads' [128, 1024] PSUM block,
  - attn@v accumulates rolling per k-chunk with a ones-column appended to v
    producing the softmax denominator in row 64,
  - the softmax divide happens AFTER the A2A on the receive side: each core
    sends raw o^T plus the denominator row per head; the receiver
    reciprocates all 16 denominators in one partition-parallel DVE op,
    broadcasts via a DRAM-bounce DMA, and multiplies once per half. This
    keeps the entire divide chain off the kernel tail,
  - AllToAll halves are "diagonal": half A = {qt0, qt3}, half B = {qt1, qt2}
    (each covers all 8 destination cores). Half B completes at 3/4 of the
    batch so its collective and projection overlap the remaining attention
    units; only half A's 0.26 MB collective remains at the batch end. Cores
    0-3 receive row-block 0 from half A, cores 4-7 row-block 1 (host gather
    compensates),
  - batch-1 qkv projection / output projection matmul chains are dribbled
    into the attention k-chunk loop so the in-order PE stream never starves
    the exp pipeline for long,
  - each core owns 2 x 128 rows of each batch; host reassembles.
"""

import math

import numpy as np

import concourse.bass as bass
import concourse.mybir as mybir
import concourse.tile as tile
from concourse import bacc
from concourse.bass_utils import run_bass_kernel_spmd
from concourse.masks import make_identity

B, S, E, H = 2, 2048, 1024, 16
HD = E // H  # 64
BASE = 10000.0
N_CORES = 8
HPC = H // N_CORES       # heads per core = 2
R = B * S                # 4096 flattened rows
RT = 512                 # rows per r-tile
NEC = E // 128           # 8 e-chunks of 128
QT = 512                 # q columns per q-tile
N_QT = S // QT           # 4 q-tiles per batch
KC = 128                 # k rows per k-chunk
N_KC = S // KC           # 16 k-chunks per batch
RPB = S // N_CORES       # rows per core per batch = 256

F32 = mybir.dt.float32
BF16 = mybir.dt.bfloat16
FP8 = mybir.dt.float8e4
EXPF = mybir.ActivationFunctionType.Exp
DROW = mybir.MatmulPerfMode.DoubleRow
# softmax numerator/denominator are both scaled by 1/EXPC (ratio unchanged);
# keeps exp(s)/EXPC <= 240 (TRN fp8e4 max) for any realistic score
EXPC = 4.0

# diagonal A2A halves: each covers all 8 destination cores
HALF = {0: "A", 1: "B", 2: "B", 3: "A"}

_COMPILED = {}


def _build_nc():
    nc = bacc.Bacc("TRN2", target_bir_lowering=False, debug=False,
                   num_devices=N_CORES)

    xT = nc.dram_tensor("xT", [E, R], BF16, kind="ExternalInput").ap()
    wqT = nc.dram_tensor("wqT", [E, 128], BF16, kind="ExternalInput").ap()
    wkT = nc.dram_tensor("wkT", [E, 128], BF16, kind="ExternalInput").ap()
    wvT = nc.dram_tensor("wvT", [E, 128], BF16, kind="ExternalInput").ap()
    w2T = nc.dram_tensor("w2T", [E, E], BF16, kind="ExternalInput").ap()
    cosT = nc.dram_tensor("cosT", [128, S], F32, kind="ExternalInput").ap()
    sinT = nc.dram_tensor("sinT", [128, S], F32, kind="ExternalInput").ap()
    p2T = nc.dram_tensor("p2T", [128, 128], BF16, kind="ExternalInput").ap()
    out = nc.dram_tensor("out", [2 * RPB, E], F32, kind="ExternalOutput").ap()

    with tile.TileContext(nc) as tc:
        _emit(tc, nc, xT, wqT, wkT, wvT, w2T, cosT, sinT, p2T, out)
    nc.compile()
    return nc


def _emit(tc, nc, xT, wqT, wkT, wvT, w2T, cosT, sinT, p2T, out):
    import contextlib
    ctx = contextlib.ExitStack()
    consts = ctx.enter_context(tc.tile_pool(name="consts", bufs=1))
    xtp = ctx.enter_context(tc.tile_pool(name="xtp", bufs=2))
    qkp = ctx.enter_context(tc.tile_pool(name="qkp", bufs=1))
    rawp = ctx.enter_context(tc.tile_pool(name="rawp", bufs=2))
    tmpp = ctx.enter_context(tc.tile_pool(name="tmpp", bufs=2))
    vp = ctx.enter_context(tc.tile_pool(name="vp", bufs=1))
    pp = ctx.enter_context(tc.tile_pool(name="pp", bufs=7))
    smallp = ctx.enter_context(tc.tile_pool(name="smallp", bufs=2))
    dramp = ctx.enter_context(tc.tile_pool(name="dramp", bufs=1, space="DRAM"))
    # PSUM budget (8 banks): qkv-shared 2 + sps 2 x 2 + av 2 = 8
    ps_qkv = ctx.enter_context(tc.tile_pool(name="ps_qkv", bufs=2, space="PSUM"))
    ps_sps = ctx.enter_context(tc.tile_pool(name="ps_sps", bufs=2, space="PSUM"))
    ps_av = ctx.enter_context(tc.tile_pool(name="ps_av", bufs=2, space="PSUM"))

    # ---- bulk input loads on the gpsimd SWDGE queue (parallel transfers);
    # with xt bufs=8 nothing waits in-FIFO, so the collective triggers
    # emitted later are never delayed ----
    wq_all = consts.tile([128, NEC, 128], BF16, tag="wq", name="wq_all")
    nc.sync.dma_start(out=wq_all[:], in_=wqT.rearrange("(c p) f -> p c f", p=128))

    xts = []
    for rt in range(2 * N_QT):
        t = xtp.tile([128, NEC, RT], BF16, tag="xt", bufs=8, name=f"xt_{rt}")
        nc.gpsimd.dma_start(
            out=t[:],
            in_=xT.rearrange("(c p) r -> p c r", p=128)[:, :, rt * RT:(rt + 1) * RT])
        xts.append(t)
        if rt == 0:
            wk_all = consts.tile([128, NEC, 128], BF16, tag="wk", name="wk_all")
            nc.gpsimd.dma_start(
                out=wk_all[:], in_=wkT.rearrange("(c p) f -> p c f", p=128))
            wv_all = consts.tile([128, NEC, 128], BF16, tag="wv", name="wv_all")
            nc.gpsimd.dma_start(
                out=wv_all[:], in_=wvT.rearrange("(c p) f -> p c f", p=128))
            p2_sb = consts.tile([128, 128], BF16, tag="p2", name="p2_sb")
            nc.gpsimd.dma_start(out=p2_sb[:], in_=p2T[:, :])
        if rt == 1:
            cos_sb = consts.tile([128, S], F32, tag="cos", name="cos_sb")
            nc.gpsimd.dma_start(out=cos_sb[:], in_=cosT[:, :])
            sin_sb = consts.tile([128, S], F32, tag="sin", name="sin_sb")
            nc.gpsimd.dma_start(out=sin_sb[:], in_=sinT[:, :])
    # w2: 2 MB, overlaps the batch-0 qkv/attention stretch
    w2_all = consts.tile([128, NEC, E], BF16, tag="w2", name="w2_all")
    nc.gpsimd.dma_start(out=w2_all[:], in_=w2T.rearrange("(c p) f -> p c f", p=128))

    ones_f32 = consts.tile([128, 64], F32, tag="ones32", name="ones_f32")
    nc.vector.memset(ones_f32[:], 1.0)
    ones_rb = consts.tile([1, 64], BF16, tag="onesrb", name="ones_rb")
    nc.vector.tensor_copy(ones_rb[:], ones_f32[0:1, 0:64])
    id_sb = consts.tile([128, 128], F32, tag="idm", name="id_sb")
    make_identity(nc, id_sb[:])

    # A2A buffers, one pair per (batch, half): dest core j's chunk is
    # [2 heads, 65 rows (64 o^T + denominator), 128 s-cols]
    send_d = {(b, hf): dramp.tile([N_CORES, HPC, 65, 128], BF16,
                                  name=f"send{b}{hf}")
              for b in range(B) for hf in ("A", "B")}
    recv_d = {(b, hf): dramp.tile([N_CORES, HPC, 65, 128], BF16,
                                  name=f"recv{b}{hf}")
              for b in range(B) for hf in ("A", "B")}

    qT_sb, kT_sb, v_sb = {}, {}, {}

    def qkv_chains(rt):
        """Return a list of closures, each emitting one matmul chain (+ its
        epilogue) for r-tile rt. Callers dribble these between attention
        steps to keep the in-order PE stream dense but never monolithic."""
        b, st = rt // N_QT, (rt % N_QT) * RT
        xt = xts[rt]

        if b not in qT_sb:
            qT_sb[b] = qkp.tile([128, S], BF16, tag=f"qT{b}", name=f"qT{b}")
            kT_sb[b] = qkp.tile([128, S], BF16, tag=f"kT{b}", name=f"kT{b}")

        def qk_chain(kind, w_all, dst):
            state = {}
            def emit_a():
                acc = ps_qkv.tile([128, RT], F32, tag="qkv",
                                  name=f"{kind}acc{rt}")
                for ec in range(4):
                    nc.tensor.matmul(acc[:], w_all[:, ec, :], xt[:, ec, :],
                                     start=(ec == 0), stop=False)
                state["acc"] = acc
            def emit_b():
                acc = state.pop("acc")
                for ec in range(4, NEC):
                    nc.tensor.matmul(acc[:], w_all[:, ec, :], xt[:, ec, :],
                                     start=False, stop=(ec == NEC - 1))
                raw = rawp.tile([128, RT], BF16, tag="raw",
                                name=f"{kind}raw{rt}")
                # DVE eviction: keep the Scalar engine free for exp, which
                # paces the attention phases these chains dribble into
                nc.vector.tensor_copy(raw[:], acc[:])
                rot = ps_qkv.tile([128, RT], F32, tag="qkv",
                                  name=f"{kind}rot{rt}")
                nc.tensor.matmul(rot[:], p2_sb[:], raw[:], start=True, stop=True)
                t1 = tmpp.tile([128, RT], F32, tag="ropet", name=f"{kind}t1_{rt}")
                nc.vector.tensor_mul(t1[:], raw[:], cos_sb[:, st:st + RT])
                t2 = tmpp.tile([128, RT], F32, tag="ropet", name=f"{kind}t2_{rt}")
                nc.vector.tensor_mul(t2[:], rot[:], sin_sb[:, st:st + RT])
                nc.vector.tensor_add(dst[:, st:st + RT], t1[:], t2[:])
            return [emit_a, emit_b]

        vstate = {}

        def v_head_chain(half):
            # v^T = wv.T @ x computed at full rate (N=512), half the e-chunks
            # per pop; the PE transpose in v_tail_chain flips it back to the
            # [k, hd] layout attn@v needs.
            def emit():
                if half == 0:
                    vacc = ps_qkv.tile([128, RT], F32, tag="qkv",
                                       name=f"vTacc{rt}")
                    vstate["ps"] = vacc
                vacc = vstate["ps"]
                for ec in range(4 * half, 4 * half + 4):
                    nc.tensor.matmul(vacc[:], wv_all[:, ec, :], xt[:, ec, :],
                                     start=(ec == 0), stop=(ec == NEC - 1))
                if half == 1:
                    vts = rawp.tile([128, RT], F32, tag="raw",
                                    name=f"vts{rt}")
                    nc.vector.tensor_copy(vts[:], vstate.pop("ps")[:])
                    vstate["sb"] = vts
            return emit

        def v_tail_chain(pair):
            def emit():
                vts = vstate["sb"]
                for sub in (2 * pair, 2 * pair + 1):
                    vtr = ps_qkv.tile([128, 128], F32, tag="qkv",
                                      name=f"vtr{rt}_{sub}")
                    nc.tensor.transpose(
                        vtr[:], vts[:, sub * 128:(sub + 1) * 128], id_sb[:])
                    kc = (rt % N_QT) * 4 + sub
                    for h in range(HPC):
                        vt = vp.tile([128, 65], BF16, tag=f"v{b}{h}{kc}",
                                     name=f"v{b}{h}{kc}")
                        nc.vector.tensor_copy(vt[:, 0:64],
                                              vtr[:, h * 64:(h + 1) * 64])
                        nc.vector.tensor_copy(vt[:, 64:65], ones_f32[:, 0:1])
                        v_sb[(b, h, kc)] = vt
            return emit

        return qk_chain("q", wq_all, qT_sb[b]) + \
               qk_chain("k", wk_all, kT_sb[b]) + \
               [v_head_chain(0), v_head_chain(1),
                v_tail_chain(0), v_tail_chain(1)]

    def proj_chains(b, hf):
        """Output projection for my 128 rows of (batch b, diagonal half hf).
        The recv load + softmax divide are emitted lazily by the first chain
        so they never precede the collective's emission."""
        state0 = {}
        def get_odv():
            if "odv" not in state0:
                # o^T rows: e-row within source chunk c is h*64+p
                recv_sb = xtp.tile([128, NEC, 128], BF16, tag="recv", bufs=2,
                                   name=f"recv{b}{hf}")
                for h in range(HPC):
                    nc.gpsimd.dma_start(
                        out=recv_sb[h * 64:(h + 1) * 64, :, :],
                        in_=recv_d[(b, hf)][:, h, 0:64, :].rearrange(
                            "c p r -> p c r"))
                # denominator rows, one partition per (source, head)
                dn = smallp.tile([16, 128], BF16, tag="dn", name=f"dn{b}{hf}")
                nc.gpsimd.dma_start(
                    out=dn[:],
                    in_=recv_d[(b, hf)][:, :, 64:65, :].rearrange(
                        "c h p r -> (c h p) r"))
                # all 16 reciprocals in one partition-parallel op (~0.9us)
                rcp16 = smallp.tile([16, 128], BF16, tag="rcp16",
                                    name=f"rcp16{b}{hf}")
                with nc.allow_low_precision(reason="bf16 1/denominator"):
                    nc.vector.reciprocal(rcp16[:], dn[:])
                # stage 1/d in DRAM as [h, c*128+r] so every broadcast read
                # below is one contiguous 2 KB row, not strided 256 B pieces
                rcp_dr = dramp.tile([HPC, NEC * 128], BF16, tag="rcpd",
                                    bufs=2, name=f"rcpd{b}{hf}")
                st_ap = bass.AP(tensor=rcp_dr.tensor, offset=rcp_dr.offset,
                                ap=[[128, N_CORES], [NEC * 128, HPC],
                                    [1, 128]])
                nc.sync.dma_start(out=st_ap, in_=rcp16[:])
                odv = xtp.tile([128, NEC, 128], BF16, tag="odv", bufs=2,
                               name=f"odv{b}{hf}")
                if b == 1:
                    # tail halves: the attention stream is over, so ps_sps
                    # and the PE are free — broadcast 1/d with one K=1
                    # matmul per head (the replicating-DMA bounce costs
                    # ~10us of strided small reads; this is ~1us)
                    dnr = []
                    for h in range(HPC):
                        t = smallp.tile([1, NEC * 128], BF16, tag=f"dnr{h}",
                                        name=f"dnr{b}{hf}{h}")
                        nc.sync.dma_start(out=t[:], in_=rcp_dr[h:h + 1, :])
                        dnr.append(t)
                    bc_ps = ps_sps.tile([128, NEC * 128], F32, tag="sps",
                                        name=f"bcps{b}{hf}")
                    for h in range(HPC):
                        for half in range(2):
                            sl = slice(half * 512, (half + 1) * 512)
                            nc.tensor.matmul(bc_ps[h * 64:(h + 1) * 64, sl],
                                             ones_rb[:], dnr[h][:, sl],
                                             start=True, stop=True)
                    nc.vector.tensor_mul(
                        odv[:], recv_sb[:],
                        bc_ps[:].rearrange("p (c r) -> p c r", c=NEC))
                else:
                    # mid-stream halves: latency is hidden by the attention
                    # stream — broadcast via the DRAM bounce (each partition
                    # reads its head's full 2 KB row)
                    bcast = smallp.tile([128, NEC, 128], BF16, tag="bcast",
                                        name=f"bcast{b}{hf}")
                    for h in range(HPC):
                        bc_ap = bass.AP(
                            tensor=rcp_dr.tensor,
                            offset=rcp_dr.offset + h * NEC * 128,
                            ap=[[0, 64], [128, NEC], [1, 128]])
                        nc.sync.dma_start(out=bcast[h * 64:(h + 1) * 64, :, :],
                                          in_=bc_ap)
                    nc.vector.tensor_mul(odv[:], recv_sb[:], bcast[:])
                state0["odv"] = odv
            return state0["odv"]
        chains = []
        rblk = {"A": 0, "B": 1}[hf]     # for cores 0-3; host swaps for 4-7
        for ft in range(2):
            state = {}
            def emit_a(ft=ft, state=state):
                odv = get_odv()
                # qkv psum tag: free during attention (projection is done)
                ops = ps_qkv.tile([128, 512], F32, tag="qkv",
                                  name=f"ops{b}_{rblk}_{ft}")
                for ec in range(4):
                    nc.tensor.matmul(
                        ops[:],
                        odv[:, ec, :],
                        w2_all[:, ec, ft * 512:(ft + 1) * 512],
                        start=(ec == 0), stop=False)
                state["ops"] = ops
            def emit_b(ft=ft, state=state):
                odv = get_odv()
                ops = state.pop("ops")
                for ec in range(4, NEC):
                    nc.tensor.matmul(
                        ops[:],
                        odv[:, ec, :],
                        w2_all[:, ec, ft * 512:(ft + 1) * 512],
                        start=False, stop=(ec == NEC - 1))
                ot = tmpp.tile([128, 512], F32, tag="ropet",
                               name=f"ot{b}_{rblk}_{ft}")
                if b == 1 and hf == "A":
                    # kernel tail: exp stream is over, ACT is free
                    nc.scalar.copy(ot[:], ops[:])
                else:
                    # runs during an attention stretch where exp keeps ACT
                    # busy: evict on DVE
                    nc.vector.tensor_copy(ot[:], ops[:])
                ob = 2 * b + rblk
                nc.sync.dma_start(
                    out=out[ob * 128:(ob + 1) * 128,
                            ft * 512:(ft + 1) * 512],
                    in_=ot[:])
            chains.append(emit_a)
            chains.append(emit_b)
        return chains

    def emit_stage(b, qt, avs):
        """Evict the attn@v accumulator (o^T raw + denominator row) straight
        into the A2A send buffer; the divide happens on the receive side."""
        last = (b == B - 1 and qt == N_QT - 1)
        hf = HALF[qt]
        jbase = 4 * (qt % 2)
        for h in range(HPC):
            # evict immediately: releases the PSUM slot so the next q-tile's
            # attn@v never waits
            oraw = smallp.tile([65, QT], BF16, tag="oraw",
                               name=f"oraw{b}{h}{qt}")
            if last:
                nc.scalar.copy(oraw[:], avs[h][:])
            else:
                # exp paces the attention stream: keep evictions off ACT
                nc.vector.tensor_copy(oraw[:], avs[h][:])
            # at the kernel tail these sends gate the final A2A: split them
            # across both HWDGE queues (ACT is idle there)
            eng = nc.scalar if (last and h == 1) else nc.sync
            for jj in range(4):
                eng.dma_start(
                    out=send_d[(b, hf)][jbase + jj, h, :, :],
                    in_=oraw[:, jj * 128:(jj + 1) * 128])

    def emit_attention_batch(b, dribble):
        """All 4 q-tiles of a batch as one rolling pipeline over 64+LAG
        (qt, kc) units: scores+exp lead, attn@v trails by LAG units, the
        staging fires as each q-tile's accumulation completes. One dribble
        chain (qkv projection / output projection) is popped every other
        unit to keep the in-order PE stream dense."""
        scale = 1.0 / math.sqrt(HD)
        NU = N_QT * N_KC
        LAG = 5
        pts = {}
        avs = {}
        for u in range(NU + LAG):
            if u < NU:
                qt, kc = divmod(u, N_KC)
                if kc == 0:
                    avs[qt] = [ps_av.tile([65, QT], F32, tag="av",
                                          name=f"av{b}{h}{qt}")
                               for h in range(HPC)]
                sps = ps_sps.tile([128, 2 * QT], F32, tag="sps",
                                  name=f"s{b}{qt}_{kc}")
                for h in range(HPC):
                    hof = h * 64
                    nc.tensor.matmul(
                        sps[:, h * QT:(h + 1) * QT],
                        kT_sb[b][hof:hof + 64, kc * KC:(kc + 1) * KC],
                        qT_sb[b][hof:hof + 64, qt * QT:(qt + 1) * QT],
                        start=True, stop=True)
                pt = pp.tile([128, 2 * QT], BF16, tag="p", name=f"p{b}{qt}_{kc}")
                nc.scalar.activation(pt[:], sps[:], EXPF, scale=scale)
                pts[u] = pt
            if u >= LAG:
                j = u - LAG
                qt2, kc2 = divmod(j, N_KC)
                for h in range(HPC):
                    nc.tensor.matmul(avs[qt2][h][:], v_sb[(b, h, kc2)][:],
                                     pts[j][:, h * QT:(h + 1) * QT],
                                     start=(kc2 == 0), stop=(kc2 == N_KC - 1))
                del pts[j]
                if kc2 == N_KC - 1:
                    emit_stage(b, qt2, avs.pop(qt2))
                    if qt2 == 2:
                        emit_a2a(b, "B")
            # one chain per two units, ramping up near the end so no
            # backlog remains to run as a monolithic lump afterwards
            if dribble and dribble[0][0] <= u and (
                    u % 2 == 1 or 2 * len(dribble) >= (NU + LAG - u)):
                dribble.pop(0)[1]()

    def emit_a2a(b, hf):
        nc.gpsimd.collective_compute(
            "AllToAll", mybir.AluOpType.bypass,
            replica_groups=[list(range(N_CORES))],
            ins=[send_d[(b, hf)].opt()], outs=[recv_d[(b, hf)].opt()])

    # ---------------- emission ----------------
    for rt in range(N_QT):             # batch-0 projection: pure PE stretch
        for chain in qkv_chains(rt):
            chain()
    # warm the collective path (cold-start ~8us); emitted here so the wait on
    # the gpsimd queue never delays the critical first x/weight loads
    cwu_s = dramp.tile([N_CORES, 8], F32, tag="cwus", name="cwu_s")
    cwu_r = dramp.tile([N_CORES, 8], F32, tag="cwur", name="cwu_r")
    nc.sync.dma_start(out=cwu_s.rearrange("c r -> (c r)")[None, :],
                      in_=ones_f32[0:1, 0:64])
    nc.gpsimd.collective_compute(
        "AllToAll", mybir.AluOpType.bypass,
        replica_groups=[list(range(N_CORES))],
        ins=[cwu_s.opt()], outs=[cwu_r.opt()])

    # batch-0 attention with batch-1 qkv dribbled in; A2A(0,B) fires at 3/4
    dribble = [(1, c) for rt in range(N_QT, 2 * N_QT) for c in qkv_chains(rt)]
    emit_attention_batch(0, dribble)
    for _, chain in dribble:
        chain()
    del dribble[:]
    emit_a2a(0, "A")                   # fires at batch-0 end

    # batch-1 attention: batch-0 projections early (both its A2As have
    # landed). Batch-1's own projections run post-loop: their PE matmuls
    # wait on collectives, and dribbling them would fence the in-order PE
    # stream mid-attention. A2A(1,A) is emitted first so its trigger fires
    # the moment the qt3 sends land.
    dribble = [(5, c) for c in proj_chains(0, "B")]
    dribble += [(40, c) for c in proj_chains(0, "A")]
    emit_attention_batch(1, dribble)
    for _, chain in dribble:
        chain()
    emit_a2a(1, "A")
    for chain in proj_chains(1, "B"):
        chain()
    for chain in proj_chains(1, "A"):
        chain()
    ctx.close()


def _host_prep(x, w1, w2):
    import ml_dtypes
    bf16 = ml_dtypes.bfloat16
    x = np.asarray(x, dtype=np.float32)
    w1 = np.asarray(w1, dtype=np.float32)
    w2 = np.asarray(w2, dtype=np.float32)

    xT = np.ascontiguousarray(x.reshape(R, E).T.astype(bf16))      # [E, R]
    w2T = np.ascontiguousarray(w2.T.astype(bf16))                  # [E, E]

    theta = 1.0 / (BASE ** (np.arange(0, HD, 2, dtype=np.float32) / HD))
    enc = np.arange(S, dtype=np.float32)[:, None] * theta[None, :]
    enc = np.repeat(enc, 2, axis=-1)                      # [s, 64]
    cos1 = np.cos(enc).T.astype(np.float32)               # [64, S]
    sin1 = np.sin(enc).T.astype(np.float32)
    cosT = np.ascontiguousarray(np.concatenate([cos1, cos1], axis=0))
    sinT = np.ascontiguousarray(np.concatenate([sin1, sin1], axis=0))

    m64 = np.zeros((HD, HD), dtype=np.float32)
    for i in range(HD // 2):
        m64[2 * i, 2 * i + 1] = -1.0
        m64[2 * i + 1, 2 * i] = 1.0
    m128 = np.zeros((128, 128), dtype=np.float32)
    m128[:64, :64] = m64
    m128[64:, 64:] = m64
    p2T = np.ascontiguousarray(m128.T.astype(bf16))

    in_maps = []
    for c in range(N_CORES):
        hA, hB = HPC * c, HPC * c + 1
        def rows(base):
            return np.concatenate(
                [w1[base + hA * HD: base + (hA + 1) * HD, :],
                 w1[base + hB * HD: base + (hB + 1) * HD, :]], axis=0)
        in_maps.append({
            "xT": xT,
            "wqT": np.ascontiguousarray(rows(0).T.astype(bf16)),
            "wkT": np.ascontiguousarray(rows(E).T.astype(bf16)),
            "wvT": np.ascontiguousarray(rows(2 * E).T.astype(bf16)),
            "w2T": w2T,
            "cosT": cosT,
            "sinT": sinT,
            "p2T": p2T,
        })
    return in_maps


def kernel(x, w1, w2, _trace=False):
    if "nc" not in _COMPILED:
        _COMPILED["nc"] = _build_nc()
    nc = _COMPILED["nc"]
    in_maps = _host_prep(x, w1, w2)
    res = run_bass_kernel_spmd(nc, in_maps, core_ids=list(range(N_CORES)),
                               trace=_trace)
    _COMPILED["last_result"] = res
    # core c returns [512, E] as four 128-row blocks written per (batch,
    # half): [b0 A, b0 B, b1 A, b1 B]. Half A carries s-rows 128c for cores
    # 0-3 but 1024+128c for cores 4-7 (diagonal halves); B is the opposite.
    full = np.empty((B, S, E), dtype=np.float32)
    for c in range(N_CORES):
        blk = res.results[c]["out"]
        lo, hi = (0, 1) if c < 4 else (1, 0)   # blk index carrying s=128c
        for b in range(B):
            full[b, 128 * c:128 * (c + 1)] = blk[(2 * b + lo) * 128:
                                                 (2 * b + lo + 1) * 128]
            full[b, 1024 + 128 * c:1024 + 128 * (c + 1)] = \
                blk[(2 * b + hi) * 128:(2 * b + hi + 1) * 128]
    return full
